# revision 1
# baseline (speedup 1.0000x reference)
"""CombinedMarginLoss (ArcFace branch, m1=1, m2=0.5, m3=0) on 8 Trainium2 cores.

Math: out[b,c] = 64 * logits[b,c] everywhere except the label column of each
row, where out = 64 * cos(arccos(clip(x)) + 0.5).  The trig expands to
x*cos(.5) - sqrt(1-x^2)*sin(.5), so no transcendental sweep is needed: the
bulk of the tensor is a pure scale-by-64 stream, and only the 128 (row, label)
elements need the margin transform.

Sharding (PartialFC style): split num_classes across the 8 cores; each core
streams its [128, 125000] shard through SBUF (DMA in -> x64 on ACT -> DMA out)
and fixes up the label columns it owns with a tiny indirect-DMA
gather/compute/scatter on the side.

Written in raw Bass (explicit semaphores, standalone wait_ge instructions):
the walrus build in this toolchain rejects any instruction carrying more than
one sync wait, which rules out the Tile scheduler's emitted sync_info.
"""

import math
from contextlib import ExitStack

import numpy as np

try:
    from concourse import bass, mybir
except ImportError:  # repo not on sys.path in a fresh grading dir
    import sys

    sys.path.insert(0, "/opt/trn_rl_repo")
    from concourse import bass, mybir

from concourse.bass_utils import run_bass_kernel_spmd

B = 128
C = 1_000_000
NCORES = 8
CS = C // NCORES  # classes per core
S = 64.0
M2 = 0.5
COSM = math.cos(M2)
SINM = math.sin(M2)
F32 = mybir.dt.float32
I32 = mybir.dt.int32

TILE_W = 12500  # bulk tile width (columns); [128, W] f32 = 6.4 MB per DMA
NBUF = 4
NLANES = 4  # DMA-completion semaphore lanes, round-robin like Tile's DMAHW0-7


def default_widths(cs: int, w: int) -> list[int]:
    """Tile widths with tapered edges: small tiles at the start so the
    out-stream ramps up sooner, and at the end so the tail drains faster."""
    taper = [w // 4, w // 4, w // 2]
    if cs <= 3 * w or w % 4:
        return [min(w, cs - i * w) for i in range((cs + w - 1) // w)]
    body = cs - 2 * w  # one w of taper on each side
    n_body = body // w
    rem = body - n_body * w
    widths = taper + [w] * n_body + ([rem] if rem else []) + taper[::-1]
    assert sum(widths) == cs
    return widths


def build_program(
    cs: int = CS,
    w: int = TILE_W,
    nbuf: int = NBUF,
    repeat: int = 1,
    widths: list[int] | None = None,
    probe: str | None = None,  # None | "copy" (skip mul+fixup) | "read" (in only)
    split_mul: bool = False,  # odd tiles scaled on DVE instead of ACT
    one_ring: bool = False,  # all DMAs on the sync HWDGE ring (R/W bursts)
    burst: int = 0,  # >0: alternate pure-read / pure-write bursts of this many
    #                  tiles (needs nbuf >= 2*burst, n_tiles % burst == 0)
) -> bass.Bass:
    """repeat>1 replays the whole pipeline back-to-back into the same output
    (benchmarking aid: wall(R)-wall(1) isolates kernel time from dispatch
    overhead).  Cross-repeat races are benign: every repeat writes identical
    values, and the final scatter is ordered after all bulk writes."""
    if widths is None:
        widths = default_widths(cs, w)
    assert sum(widths) == cs and max(widths) <= w
    offsets = [0]
    for wd in widths:
        offsets.append(offsets[-1] + wd)
    n_tiles = len(widths)
    nc = bass.Bass()
    x = nc.declare_dram_parameter("x", [B, cs], F32, isOutput=False)
    idx = nc.declare_dram_parameter("idx", [B, 1], I32, isOutput=False)
    own = nc.declare_dram_parameter("own", [B, 1], F32, isOutput=False)
    y = nc.declare_dram_parameter("y", [B, cs], F32, isOutput=True)

    ALU = mybir.AluOpType
    ACTF = mybir.ActivationFunctionType

    with ExitStack() as ctx:
        bufs = [
            ctx.enter_context(nc.sbuf_tensor(f"buf{k}", [B, w], F32))
            for k in range(nbuf)
        ]
        idx_t = ctx.enter_context(nc.sbuf_tensor("idx_t", [B, 1], I32))
        own_t = ctx.enter_context(nc.sbuf_tensor("own_t", [B, 1], F32))
        xt = ctx.enter_context(nc.sbuf_tensor("xt", [B, 1], F32))
        xc = ctx.enter_context(nc.sbuf_tensor("xc", [B, 1], F32))
        sq = ctx.enter_context(nc.sbuf_tensor("sq", [B, 1], F32))
        rt = ctx.enter_context(nc.sbuf_tensor("rt", [B, 1], F32))
        t1 = ctx.enter_context(nc.sbuf_tensor("t1", [B, 1], F32))
        fx = ctx.enter_context(nc.sbuf_tensor("fx", [B, 1], F32))
        dl = ctx.enter_context(nc.sbuf_tensor("dl", [B, 1], F32))
        sm = ctx.enter_context(nc.sbuf_tensor("sm", [B, 1], F32))
        val = ctx.enter_context(nc.sbuf_tensor("val", [B, 1], F32))

        block = ctx.enter_context(nc.Block())
        in_sems = [
            ctx.enter_context(nc.semaphore(f"in_sem{k}")) for k in range(NLANES)
        ]
        out_sems = [
            ctx.enter_context(nc.semaphore(f"out_sem{k}")) for k in range(NLANES)
        ]
        fix_sem = ctx.enter_context(nc.semaphore("fix_sem"))
        dve_sem = ctx.enter_context(nc.semaphore("dve_sem"))
        act_sem = ctx.enter_context(nc.semaphore("act_sem"))
        scat_sem = ctx.enter_context(nc.semaphore("scat_sem"))
        dvb_sem = ctx.enter_context(nc.semaphore("dvb_sem"))
        fsq_sem = ctx.enter_context(nc.semaphore("fsq_sem"))

        def col_slice(i):
            return slice(offsets[i], offsets[i + 1])

        def width(i):
            return widths[i]

        # in-DMA i signals in_sems[i % NLANES]; the m-th DMA on a lane raises
        # it to 16*(m+1).  Likewise for out-DMAs.
        def lane_count(i):
            return i // NLANES + 1

        APR = 2 + n_tiles  # act_sem increments per repeat

        if burst:
            assert nbuf >= 2 * burst and n_tiles % burst == 0

        @block.sync
        def _(sync: bass.BassEngine):
            for g in range(repeat * n_tiles):
                i = g % n_tiles
                if g >= nbuf:
                    j = g - nbuf  # previous tenant of this buffer
                    recycle = in_sems if probe == "read" else out_sems
                    sync.wait_ge(recycle[j % NLANES], 16 * lane_count(j))
                if burst and g % burst == 0 and g >= burst:
                    # read-burst b starts only after write-burst b-1 drained
                    sync.wait_ge(out_sems[(g - 1) % NLANES], 16 * lane_count(g - 1))
                sync.dma_start(
                    out=bufs[g % nbuf][:, : width(i)], in_=x[:, col_slice(i)]
                ).then_inc(in_sems[g % NLANES], 16)
            if probe == "read":  # drain before program end
                G = repeat * n_tiles
                for k in range(NLANES):
                    n_k = len([g for g in range(G) if g % NLANES == k])
                    if n_k:
                        sync.wait_ge(in_sems[k], 16 * n_k)

        if probe == "read":
            return nc

        if probe == "serial":
            # All DMAs on ONE HWDGE ring (sync), alternating whole-tile reads
            # and writes: in0,in1,out0,in2,out1,...  Probes whether avoiding
            # concurrent R/W streams beats the two-ring copy skeleton.  Pure
            # copy is per-SDMA-engine FIFO-safe without in-completion waits.
            @block.sync
            def _(sync: bass.BassEngine):
                G = repeat * n_tiles
                lead = 2
                for g in range(G + lead):
                    if g < G:
                        i = g % n_tiles
                        if g >= nbuf:
                            j = g - nbuf
                            sync.wait_ge(out_sems[j % NLANES], 16 * lane_count(j))
                        sync.dma_start(
                            out=bufs[g % nbuf][:, : width(i)], in_=x[:, col_slice(i)]
                        ).then_inc(in_sems[g % NLANES], 16)
                    if g >= lead:
                        k = g - lead
                        i = k % n_tiles
                        sync.dma_start(
                            out=y[:, col_slice(i)], in_=bufs[k % nbuf][:, : width(i)]
                        ).then_inc(out_sems[k % NLANES], 16)
                for kk in range(NLANES):
                    n_k = len([g for g in range(G) if g % NLANES == kk])
                    if n_k:
                        sync.wait_ge(out_sems[kk], 16 * n_k)

            return nc

        if probe == "copy":

            @block.scalar
            def _(scalar: bass.BassEngine):
                for r in range(repeat):
                    for i in range(n_tiles):
                        g = r * n_tiles + i
                        scalar.wait_ge(in_sems[g % NLANES], 16 * lane_count(g))
                        scalar.dma_start(
                            out=y[:, col_slice(i)], in_=bufs[g % nbuf][:, : width(i)]
                        ).then_inc(out_sems[g % NLANES], 16)

            return nc

        if one_ring:
            G = repeat * n_tiles
            lead = 2

            @block.sync
            def _(sync: bass.BassEngine):
                # Single HWDGE ring serializes whole-tile reads and writes so
                # HBM never interleaves R/W at packet granularity.  out(k)
                # gates on mul(k) (act_sem is one inc per bulk mul).
                for g in range(G + lead):
                    if g < G:
                        i = g % n_tiles
                        if g >= nbuf:
                            j = g - nbuf
                            sync.wait_ge(out_sems[j % NLANES], 16 * lane_count(j))
                        sync.dma_start(
                            out=bufs[g % nbuf][:, : width(i)], in_=x[:, col_slice(i)]
                        ).then_inc(in_sems[g % NLANES], 16)
                    if g >= lead:
                        k = g - lead
                        i = k % n_tiles
                        sync.wait_ge(act_sem, k + 1)
                        sync.dma_start(
                            out=y[:, col_slice(i)], in_=bufs[k % nbuf][:, : width(i)]
                        ).then_inc(out_sems[k % NLANES], 16)

            @block.scalar
            def _(scalar: bass.BassEngine):
                for r in range(repeat):
                    for i in range(n_tiles):
                        g = r * n_tiles + i
                        scalar.wait_ge(in_sems[g % NLANES], 16 * lane_count(g))
                        b = bufs[g % nbuf]
                        scalar.mul(b[:, : width(i)], b[:, : width(i)], S).then_inc(
                            act_sem, 1
                        )
                    # fixup: sq = xc^2 ; rt = sqrt(1 - sq)
                    scalar.wait_ge(dve_sem, 6 * r + 1)
                    scalar.activation(sq[:], xc[:], ACTF.Square).then_inc(fsq_sem, 1)
                    scalar.wait_ge(fsq_sem, 2 * r + 1)
                    scalar.activation(
                        rt[:], sq[:], ACTF.Sqrt, bias=1.0, scale=-1.0
                    ).then_inc(fsq_sem, 1)

            @block.vector
            def _(vector: bass.BassEngine):
                for r in range(repeat):
                    vector.wait_ge(fix_sem, 48 * r + 48)
                    vector.tensor_scalar(
                        out=xc[:], in0=xt[:], scalar1=-1.0, scalar2=1.0,
                        op0=ALU.max, op1=ALU.min,
                    ).then_inc(dve_sem, 1)
                    vector.wait_ge(fsq_sem, 2 * r + 2)
                    vector.tensor_scalar_mul(t1[:], rt[:], SINM).then_inc(dve_sem, 1)
                    vector.wait_ge(dve_sem, 6 * r + 2)
                    vector.tensor_scalar(
                        out=fx[:], in0=xc[:], scalar1=COSM, scalar2=t1[:, :1],
                        op0=ALU.mult, op1=ALU.subtract,
                    ).then_inc(dve_sem, 1)
                    vector.wait_ge(dve_sem, 6 * r + 3)
                    vector.tensor_scalar(
                        out=dl[:], in0=fx[:], scalar1=xc[:, :1], scalar2=None,
                        op0=ALU.subtract,
                    ).then_inc(dve_sem, 1)
                    vector.wait_ge(dve_sem, 6 * r + 4)
                    vector.tensor_scalar(
                        out=sm[:], in0=dl[:], scalar1=own_t[:, :1],
                        scalar2=xc[:, :1], op0=ALU.mult, op1=ALU.add,
                    ).then_inc(dve_sem, 1)
                    vector.wait_ge(dve_sem, 6 * r + 5)
                    vector.tensor_scalar_mul(val[:], sm[:], S).then_inc(dve_sem, 1)

            @block.gpsimd
            def _(gpsimd: bass.BassEngine):
                for r in range(repeat):
                    gpsimd.dma_start(out=idx_t[:], in_=idx[:]).then_inc(fix_sem, 16)
                    gpsimd.dma_start(out=own_t[:], in_=own[:]).then_inc(fix_sem, 16)
                    gpsimd.wait_ge(fix_sem, 48 * r + 32)
                    gpsimd.indirect_dma_start(
                        out=xt[:],
                        out_offset=None,
                        in_=x[:],
                        in_offset=bass.IndirectOffsetOnAxis(ap=idx_t[:, :1], axis=1),
                    ).then_inc(fix_sem, 16)
                    gpsimd.wait_ge(dve_sem, 6 * r + 6)
                    for k in range(NLANES):
                        n_k = len(
                            [g for g in range((r + 1) * n_tiles) if g % NLANES == k]
                        )
                        if n_k:
                            gpsimd.wait_ge(out_sems[k], 16 * n_k)
                    gpsimd.indirect_dma_start(
                        out=y[:],
                        out_offset=bass.IndirectOffsetOnAxis(ap=idx_t[:, :1], axis=1),
                        in_=val[:],
                        in_offset=None,
                    ).then_inc(scat_sem, 16)
                    gpsimd.wait_ge(scat_sem, 16 * (r + 1))

            return nc

        if split_mul:
            n_even = (n_tiles + 1) // 2
            n_odd = n_tiles // 2

            @block.scalar
            def _(scalar: bass.BassEngine):
                for r in range(repeat):
                    for i in range(n_tiles):
                        g = r * n_tiles + i
                        b = bufs[g % nbuf]
                        if i % 2 == 0:  # ACT scales even tiles
                            scalar.wait_ge(in_sems[g % NLANES], 16 * lane_count(g))
                            scalar.mul(
                                b[:, : width(i)], b[:, : width(i)], S
                            ).then_inc(act_sem, 1)
                            scalar.wait_ge(act_sem, n_even * r + i // 2 + 1)
                        else:  # DVE scaled it
                            scalar.wait_ge(dvb_sem, n_odd * r + (i + 1) // 2)
                        scalar.dma_start(
                            out=y[:, col_slice(i)], in_=b[:, : width(i)]
                        ).then_inc(out_sems[g % NLANES], 16)
                    # fixup: sq = xc^2 ; rt = sqrt(1 - sq)
                    scalar.wait_ge(dve_sem, 6 * r + 1)
                    scalar.activation(sq[:], xc[:], ACTF.Square).then_inc(fsq_sem, 1)
                    scalar.wait_ge(fsq_sem, 2 * r + 1)
                    scalar.activation(
                        rt[:], sq[:], ACTF.Sqrt, bias=1.0, scale=-1.0
                    ).then_inc(fsq_sem, 1)

            @block.vector
            def _(vector: bass.BassEngine):
                for r in range(repeat):
                    for i in range(1, n_tiles, 2):
                        g = r * n_tiles + i
                        b = bufs[g % nbuf]
                        vector.wait_ge(in_sems[g % NLANES], 16 * lane_count(g))
                        vector.tensor_scalar_mul(
                            b[:, : width(i)], b[:, : width(i)], S
                        ).then_inc(dvb_sem, 1)
                    # fixup chain (after bulk so it never stalls the muls)
                    vector.wait_ge(fix_sem, 48 * r + 48)
                    vector.tensor_scalar(
                        out=xc[:], in0=xt[:], scalar1=-1.0, scalar2=1.0,
                        op0=ALU.max, op1=ALU.min,
                    ).then_inc(dve_sem, 1)
                    vector.wait_ge(fsq_sem, 2 * r + 2)
                    vector.tensor_scalar_mul(t1[:], rt[:], SINM).then_inc(dve_sem, 1)
                    vector.wait_ge(dve_sem, 6 * r + 2)
                    vector.tensor_scalar(
                        out=fx[:], in0=xc[:], scalar1=COSM, scalar2=t1[:, :1],
                        op0=ALU.mult, op1=ALU.subtract,
                    ).then_inc(dve_sem, 1)
                    vector.wait_ge(dve_sem, 6 * r + 3)
                    vector.tensor_scalar(
                        out=dl[:], in0=fx[:], scalar1=xc[:, :1], scalar2=None,
                        op0=ALU.subtract,
                    ).then_inc(dve_sem, 1)
                    vector.wait_ge(dve_sem, 6 * r + 4)
                    vector.tensor_scalar(
                        out=sm[:], in0=dl[:], scalar1=own_t[:, :1],
                        scalar2=xc[:, :1], op0=ALU.mult, op1=ALU.add,
                    ).then_inc(dve_sem, 1)
                    vector.wait_ge(dve_sem, 6 * r + 5)
                    vector.tensor_scalar_mul(val[:], sm[:], S).then_inc(dve_sem, 1)

            @block.gpsimd
            def _(gpsimd: bass.BassEngine):
                for r in range(repeat):
                    gpsimd.dma_start(out=idx_t[:], in_=idx[:]).then_inc(fix_sem, 16)
                    gpsimd.dma_start(out=own_t[:], in_=own[:]).then_inc(fix_sem, 16)
                    gpsimd.wait_ge(fix_sem, 48 * r + 32)
                    gpsimd.indirect_dma_start(
                        out=xt[:],
                        out_offset=None,
                        in_=x[:],
                        in_offset=bass.IndirectOffsetOnAxis(ap=idx_t[:, :1], axis=1),
                    ).then_inc(fix_sem, 16)
                    gpsimd.wait_ge(dve_sem, 6 * r + 6)
                    for k in range(NLANES):
                        n_k = len(
                            [g for g in range((r + 1) * n_tiles) if g % NLANES == k]
                        )
                        if n_k:
                            gpsimd.wait_ge(out_sems[k], 16 * n_k)
                    gpsimd.indirect_dma_start(
                        out=y[:],
                        out_offset=bass.IndirectOffsetOnAxis(ap=idx_t[:, :1], axis=1),
                        in_=val[:],
                        in_offset=None,
                    ).then_inc(scat_sem, 16)
                    gpsimd.wait_ge(scat_sem, 16 * (r + 1))

            return nc

        @block.scalar
        def _(scalar: bass.BassEngine):
            for r in range(repeat):
                # bulk: y tile = 64 * x tile.  Engines are pipelined, so every
                # same-engine RAW pair also gets an explicit sem sync.  The two
                # fixup ACT ops are tucked in after tile 0's out-DMA so they
                # never stall the pipeline head waiting for the SWDGE gather.
                for i in range(n_tiles):
                    g = r * n_tiles + i
                    scalar.wait_ge(in_sems[g % NLANES], 16 * lane_count(g))
                    b = bufs[g % nbuf]
                    scalar.mul(b[:, : width(i)], b[:, : width(i)], S).then_inc(
                        act_sem, 1
                    )
                    scalar.wait_ge(act_sem, APR * r + 1 + i + (2 if i > 0 else 0))
                    if burst and g % burst == 0:
                        # write-burst waits until its whole read-burst landed
                        gl = g + burst - 1
                        scalar.wait_ge(in_sems[gl % NLANES], 16 * lane_count(gl))
                    scalar.dma_start(
                        out=y[:, col_slice(i)], in_=b[:, : width(i)]
                    ).then_inc(out_sems[g % NLANES], 16)
                    if i == 0:
                        # fixup: sq = xc^2 ; rt = sqrt(1 - sq)
                        scalar.wait_ge(dve_sem, 6 * r + 1)
                        scalar.activation(sq[:], xc[:], ACTF.Square).then_inc(
                            act_sem, 1
                        )
                        scalar.wait_ge(act_sem, APR * r + 2)
                        scalar.activation(
                            rt[:], sq[:], ACTF.Sqrt, bias=1.0, scale=-1.0
                        ).then_inc(act_sem, 1)

        @block.vector
        def _(vector: bass.BassEngine):
            for r in range(repeat):
                # xc = clip(xt, -1, 1)
                vector.wait_ge(fix_sem, 48 * r + 48)
                vector.tensor_scalar(
                    out=xc[:], in0=xt[:], scalar1=-1.0, scalar2=1.0,
                    op0=ALU.max, op1=ALU.min,
                ).then_inc(dve_sem, 1)
                # after ACT's sqrt: fixed = COSM*xc - SINM*rt
                # val = S * (xc + own * (fixed - xc))
                vector.wait_ge(act_sem, APR * r + 3)
                vector.tensor_scalar_mul(t1[:], rt[:], SINM).then_inc(dve_sem, 1)
                vector.wait_ge(dve_sem, 6 * r + 2)
                vector.tensor_scalar(
                    out=fx[:], in0=xc[:], scalar1=COSM, scalar2=t1[:, :1],
                    op0=ALU.mult, op1=ALU.subtract,
                ).then_inc(dve_sem, 1)
                vector.wait_ge(dve_sem, 6 * r + 3)
                vector.tensor_scalar(
                    out=dl[:], in0=fx[:], scalar1=xc[:, :1], scalar2=None,
                    op0=ALU.subtract,
                ).then_inc(dve_sem, 1)
                vector.wait_ge(dve_sem, 6 * r + 4)
                vector.tensor_scalar(
                    out=sm[:], in0=dl[:], scalar1=own_t[:, :1], scalar2=xc[:, :1],
                    op0=ALU.mult, op1=ALU.add,
                ).then_inc(dve_sem, 1)
                vector.wait_ge(dve_sem, 6 * r + 5)
                vector.tensor_scalar_mul(val[:], sm[:], S).then_inc(dve_sem, 1)

        @block.gpsimd
        def _(gpsimd: bass.BassEngine):
            for r in range(repeat):
                gpsimd.dma_start(out=idx_t[:], in_=idx[:]).then_inc(fix_sem, 16)
                gpsimd.dma_start(out=own_t[:], in_=own[:]).then_inc(fix_sem, 16)
                gpsimd.wait_ge(fix_sem, 48 * r + 32)
                # xt[b] = x.flat[idx[b]] (flat element offset: axis=1 -> coef 1)
                gpsimd.indirect_dma_start(
                    out=xt[:],
                    out_offset=None,
                    in_=x[:],
                    in_offset=bass.IndirectOffsetOnAxis(ap=idx_t[:, :1], axis=1),
                ).then_inc(fix_sem, 16)
                # scatter val into label columns, after ALL bulk writes to y
                gpsimd.wait_ge(dve_sem, 6 * r + 6)
                for k in range(NLANES):
                    n_k = len(
                        [g for g in range((r + 1) * n_tiles) if g % NLANES == k]
                    )
                    if n_k:
                        gpsimd.wait_ge(out_sems[k], 16 * n_k)
                gpsimd.indirect_dma_start(
                    out=y[:],
                    out_offset=bass.IndirectOffsetOnAxis(ap=idx_t[:, :1], axis=1),
                    in_=val[:],
                    in_offset=None,
                ).then_inc(scat_sem, 16)
                gpsimd.wait_ge(scat_sem, 16 * (r + 1))

    return nc


_PROG = None


def _get_prog() -> bass.Bass:
    global _PROG
    if _PROG is None:
        _PROG = build_program()
    return _PROG


def make_in_maps(logits: np.ndarray, labels: np.ndarray) -> list[dict]:
    logits = np.asarray(logits, dtype=np.float32)
    labels = np.asarray(labels).astype(np.int64)
    rows = np.arange(B, dtype=np.int64)
    in_maps = []
    for m in range(NCORES):
        c0 = m * CS
        loc = labels - c0
        ownm = (labels != -1) & (loc >= 0) & (loc < CS)
        col = np.where(ownm, loc, 0)
        flat = (rows * CS + col).astype(np.int32)
        in_maps.append(
            {
                "x": np.ascontiguousarray(logits[:, c0 : c0 + CS]),
                "idx": flat.reshape(B, 1),
                "own": ownm.astype(np.float32).reshape(B, 1),
            }
        )
    return in_maps


def run(logits: np.ndarray, labels: np.ndarray, trace: bool = False):
    """Returns (full_output, BassKernelResults)."""
    in_maps = make_in_maps(logits, labels)
    res = run_bass_kernel_spmd(_get_prog(), in_maps, list(range(NCORES)), trace=trace)
    out = np.concatenate([res.results[m]["y"] for m in range(NCORES)], axis=1)
    return out, res


def kernel(logits: np.ndarray, labels: np.ndarray) -> np.ndarray:
    out, _ = run(logits, labels)
    return out



# revision 11
# speedup vs baseline: 2.0613x; 2.0613x over previous
"""CombinedMarginLoss (ArcFace branch, m1=1, m2=0.5, m3=0) on 8 Trainium2 cores.

Math: out[b,c] = 64 * logits[b,c] everywhere except the label column of each
row, where out = 64 * cos(arccos(clip(x)) + 0.5).  The trig expands to
x*cos(.5) - sqrt(1-x^2)*sin(.5), so no transcendental sweep is needed: the
bulk of the tensor is a pure scale-by-64 stream, and only the 128 (row, label)
elements need the margin transform.

Sharding (PartialFC style): split num_classes across the 8 cores; each core
streams its [128, 125000] shard through SBUF (DMA in -> x64 on ACT -> DMA out)
and fixes up the label columns it owns with a tiny indirect-DMA
gather/compute/scatter on the side.

The bulk stream runs in alternating 5-tile read / 5-tile write phases
(burst mode, w=5000, nbuf=10, soft=1): HBM sustains ~344-355 GB/s/core in a
single direction but only ~316 GB/s effective when reads and writes are
concurrently mixed, so phase alternation with a one-tile soft handoff at
each flip is ~1.5% faster than the continuously-mixed pipeline
(~393 us vs ~400 us per sweep; serialized-direction floor is ~366 us).

Written in raw Bass (explicit semaphores, standalone wait_ge instructions):
the walrus build in this toolchain rejects any instruction carrying more than
one sync wait, which rules out the Tile scheduler's emitted sync_info.
"""

import math
from contextlib import ExitStack

import numpy as np

try:
    from concourse import bass, mybir
except ImportError:  # repo not on sys.path in a fresh grading dir
    import sys

    sys.path.insert(0, "/opt/trn_rl_repo")
    from concourse import bass, mybir

from concourse.bass_utils import run_bass_kernel_spmd

B = 128
C = 1_000_000
NCORES = 8
CS = C // NCORES  # classes per core
S = 64.0
M2 = 0.5
COSM = math.cos(M2)
SINM = math.sin(M2)
F32 = mybir.dt.float32
I32 = mybir.dt.int32

TILE_W = 12500  # bulk tile width (columns); [128, W] f32 = 6.4 MB per DMA
NBUF = 4
NLANES = 4  # DMA-completion semaphore lanes, round-robin like Tile's DMAHW0-7


def default_widths(cs: int, w: int) -> list[int]:
    """Tile widths with tapered edges: small tiles at the start so the
    out-stream ramps up sooner, and at the end so the tail drains faster."""
    taper = [w // 4, w // 4, w // 2]
    if cs <= 3 * w or w % 4:
        return [min(w, cs - i * w) for i in range((cs + w - 1) // w)]
    body = cs - 2 * w  # one w of taper on each side
    n_body = body // w
    rem = body - n_body * w
    widths = taper + [w] * n_body + ([rem] if rem else []) + taper[::-1]
    assert sum(widths) == cs
    return widths


def build_program(
    cs: int = CS,
    w: int = TILE_W,
    nbuf: int = NBUF,
    repeat: int = 1,
    widths: list[int] | None = None,
    probe: str | None = None,  # None | "copy" (skip mul+fixup) | "read" (in only)
    split_mul: bool = False,  # odd tiles scaled on DVE instead of ACT
    one_ring: bool = False,  # all DMAs on the sync HWDGE ring (R/W bursts)
    burst: int = 0,  # >0: alternate pure-read / pure-write bursts of this many
    #                  tiles (needs nbuf >= 2*burst, n_tiles % burst == 0)
    soft: int = 0,  # burst mode: let the next phase start this many tiles
    #                 before the previous phase fully drains (hides the flip
    #                 bubble at the cost of brief R/W mixing)
) -> bass.Bass:
    """repeat>1 replays the whole pipeline back-to-back into the same output
    (benchmarking aid: wall(R)-wall(1) isolates kernel time from dispatch
    overhead).  Cross-repeat races are benign: every repeat writes identical
    values, and the final scatter is ordered after all bulk writes."""
    if widths is None:
        widths = default_widths(cs, w)
    assert sum(widths) == cs and max(widths) <= w
    offsets = [0]
    for wd in widths:
        offsets.append(offsets[-1] + wd)
    n_tiles = len(widths)
    nc = bass.Bass()
    x = nc.declare_dram_parameter("x", [B, cs], F32, isOutput=False)
    idx = nc.declare_dram_parameter("idx", [B, 1], I32, isOutput=False)
    own = nc.declare_dram_parameter("own", [B, 1], F32, isOutput=False)
    y = nc.declare_dram_parameter("y", [B, cs], F32, isOutput=True)

    ALU = mybir.AluOpType
    ACTF = mybir.ActivationFunctionType

    with ExitStack() as ctx:
        bufs = [
            ctx.enter_context(nc.sbuf_tensor(f"buf{k}", [B, w], F32))
            for k in range(nbuf)
        ]
        idx_t = ctx.enter_context(nc.sbuf_tensor("idx_t", [B, 1], I32))
        own_t = ctx.enter_context(nc.sbuf_tensor("own_t", [B, 1], F32))
        xt = ctx.enter_context(nc.sbuf_tensor("xt", [B, 1], F32))
        xc = ctx.enter_context(nc.sbuf_tensor("xc", [B, 1], F32))
        sq = ctx.enter_context(nc.sbuf_tensor("sq", [B, 1], F32))
        rt = ctx.enter_context(nc.sbuf_tensor("rt", [B, 1], F32))
        t1 = ctx.enter_context(nc.sbuf_tensor("t1", [B, 1], F32))
        fx = ctx.enter_context(nc.sbuf_tensor("fx", [B, 1], F32))
        dl = ctx.enter_context(nc.sbuf_tensor("dl", [B, 1], F32))
        sm = ctx.enter_context(nc.sbuf_tensor("sm", [B, 1], F32))
        val = ctx.enter_context(nc.sbuf_tensor("val", [B, 1], F32))

        block = ctx.enter_context(nc.Block())
        in_sems = [
            ctx.enter_context(nc.semaphore(f"in_sem{k}")) for k in range(NLANES)
        ]
        out_sems = [
            ctx.enter_context(nc.semaphore(f"out_sem{k}")) for k in range(NLANES)
        ]
        fix_sem = ctx.enter_context(nc.semaphore("fix_sem"))
        dve_sem = ctx.enter_context(nc.semaphore("dve_sem"))
        act_sem = ctx.enter_context(nc.semaphore("act_sem"))
        scat_sem = ctx.enter_context(nc.semaphore("scat_sem"))
        dvb_sem = ctx.enter_context(nc.semaphore("dvb_sem"))
        fsq_sem = ctx.enter_context(nc.semaphore("fsq_sem"))

        def col_slice(i):
            return slice(offsets[i], offsets[i + 1])

        def width(i):
            return widths[i]

        # in-DMA i signals in_sems[i % NLANES]; the m-th DMA on a lane raises
        # it to 16*(m+1).  Likewise for out-DMAs.
        def lane_count(i):
            return i // NLANES + 1

        APR = 2 + n_tiles  # act_sem increments per repeat

        if burst:
            assert nbuf >= 2 * burst and n_tiles % burst == 0

        if probe in ("read2", "write2"):
            # One direction only, split across BOTH HWDGE rings (sync=SP ring
            # even tiles, scalar=ACT ring odd tiles).  If a single ring is
            # the ~344 GB/s cap, this should beat the 1-ring read/write
            # probes; if the HBM bus is the cap, it will match them.
            half = nbuf // 2
            G = repeat * n_tiles

            def gen(eng, e):
                ks = [g for g in range(G) if (g % n_tiles) % 2 == e]
                for k, g in enumerate(ks):
                    i = g % n_tiles
                    lane = 2 * e + (k % 2)
                    if probe == "read2":
                        if k >= half:
                            kp = k - half
                            eng.wait_ge(
                                in_sems[2 * e + kp % 2], 16 * (kp // 2 + 1)
                            )
                        eng.dma_start(
                            out=bufs[e * half + k % half][:, : width(i)],
                            in_=x[:, col_slice(i)],
                        ).then_inc(in_sems[lane], 16)
                    else:
                        eng.dma_start(
                            out=y[:, col_slice(i)],
                            in_=bufs[e * half + k % half][:, : width(i)],
                        ).then_inc(in_sems[lane], 16)
                for off in (0, 1):
                    n_l = len([k for k in range(len(ks)) if k % 2 == off])
                    if n_l:
                        eng.wait_ge(in_sems[2 * e + off], 16 * n_l)

            @block.sync
            def _(sync: bass.BassEngine):
                gen(sync, 0)

            @block.scalar
            def _(scalar: bass.BassEngine):
                gen(scalar, 1)

            return nc

        if probe == "write":
            # Pure write stream: y tiles from (uninitialized) SBUF buffers.
            # Buffers are read-only sources, so no recycle waits are needed;
            # only a final drain before program end.  No in-DMAs at all.
            @block.scalar
            def _(scalar: bass.BassEngine):
                G = repeat * n_tiles
                for g in range(G):
                    i = g % n_tiles
                    scalar.dma_start(
                        out=y[:, col_slice(i)], in_=bufs[g % nbuf][:, : width(i)]
                    ).then_inc(out_sems[g % NLANES], 16)
                for k in range(NLANES):
                    n_k = len([g for g in range(G) if g % NLANES == k])
                    if n_k:
                        scalar.wait_ge(out_sems[k], 16 * n_k)

            return nc

        @block.sync
        def _(sync: bass.BassEngine):
            for g in range(repeat * n_tiles):
                i = g % n_tiles
                if g >= nbuf:
                    j = g - nbuf  # previous tenant of this buffer
                    recycle = in_sems if probe == "read" else out_sems
                    sync.wait_ge(recycle[j % NLANES], 16 * lane_count(j))
                if burst and g % burst == 0 and g >= burst:
                    # read-burst b starts only after write-burst b-1 drained
                    # (with soft>0: after all but `soft` of its tiles drained)
                    gw = g - 1 - soft
                    if gw >= 0:
                        sync.wait_ge(out_sems[gw % NLANES], 16 * lane_count(gw))
                sync.dma_start(
                    out=bufs[g % nbuf][:, : width(i)], in_=x[:, col_slice(i)]
                ).then_inc(in_sems[g % NLANES], 16)
            if probe == "read":  # drain before program end
                G = repeat * n_tiles
                for k in range(NLANES):
                    n_k = len([g for g in range(G) if g % NLANES == k])
                    if n_k:
                        sync.wait_ge(in_sems[k], 16 * n_k)

        if probe == "read":
            return nc

        if probe == "serial":
            # All DMAs on ONE HWDGE ring (sync), alternating whole-tile reads
            # and writes: in0,in1,out0,in2,out1,...  Probes whether avoiding
            # concurrent R/W streams beats the two-ring copy skeleton.  Pure
            # copy is per-SDMA-engine FIFO-safe without in-completion waits.
            @block.sync
            def _(sync: bass.BassEngine):
                G = repeat * n_tiles
                lead = 2
                for g in range(G + lead):
                    if g < G:
                        i = g % n_tiles
                        if g >= nbuf:
                            j = g - nbuf
                            sync.wait_ge(out_sems[j % NLANES], 16 * lane_count(j))
                        sync.dma_start(
                            out=bufs[g % nbuf][:, : width(i)], in_=x[:, col_slice(i)]
                        ).then_inc(in_sems[g % NLANES], 16)
                    if g >= lead:
                        k = g - lead
                        i = k % n_tiles
                        sync.dma_start(
                            out=y[:, col_slice(i)], in_=bufs[k % nbuf][:, : width(i)]
                        ).then_inc(out_sems[k % NLANES], 16)
                for kk in range(NLANES):
                    n_k = len([g for g in range(G) if g % NLANES == kk])
                    if n_k:
                        sync.wait_ge(out_sems[kk], 16 * n_k)

            return nc

        if probe == "copy":

            @block.scalar
            def _(scalar: bass.BassEngine):
                for r in range(repeat):
                    for i in range(n_tiles):
                        g = r * n_tiles + i
                        scalar.wait_ge(in_sems[g % NLANES], 16 * lane_count(g))
                        scalar.dma_start(
                            out=y[:, col_slice(i)], in_=bufs[g % nbuf][:, : width(i)]
                        ).then_inc(out_sems[g % NLANES], 16)

            return nc

        if one_ring:
            G = repeat * n_tiles
            lead = 2

            @block.sync
            def _(sync: bass.BassEngine):
                # Single HWDGE ring serializes whole-tile reads and writes so
                # HBM never interleaves R/W at packet granularity.  out(k)
                # gates on mul(k) (act_sem is one inc per bulk mul).
                for g in range(G + lead):
                    if g < G:
                        i = g % n_tiles
                        if g >= nbuf:
                            j = g - nbuf
                            sync.wait_ge(out_sems[j % NLANES], 16 * lane_count(j))
                        sync.dma_start(
                            out=bufs[g % nbuf][:, : width(i)], in_=x[:, col_slice(i)]
                        ).then_inc(in_sems[g % NLANES], 16)
                    if g >= lead:
                        k = g - lead
                        i = k % n_tiles
                        sync.wait_ge(act_sem, k + 1)
                        sync.dma_start(
                            out=y[:, col_slice(i)], in_=bufs[k % nbuf][:, : width(i)]
                        ).then_inc(out_sems[k % NLANES], 16)

            @block.scalar
            def _(scalar: bass.BassEngine):
                for r in range(repeat):
                    for i in range(n_tiles):
                        g = r * n_tiles + i
                        scalar.wait_ge(in_sems[g % NLANES], 16 * lane_count(g))
                        b = bufs[g % nbuf]
                        scalar.mul(b[:, : width(i)], b[:, : width(i)], S).then_inc(
                            act_sem, 1
                        )
                    # fixup: sq = xc^2 ; rt = sqrt(1 - sq)
                    scalar.wait_ge(dve_sem, 6 * r + 1)
                    scalar.activation(sq[:], xc[:], ACTF.Square).then_inc(fsq_sem, 1)
                    scalar.wait_ge(fsq_sem, 2 * r + 1)
                    scalar.activation(
                        rt[:], sq[:], ACTF.Sqrt, bias=1.0, scale=-1.0
                    ).then_inc(fsq_sem, 1)

            @block.vector
            def _(vector: bass.BassEngine):
                for r in range(repeat):
                    vector.wait_ge(fix_sem, 48 * r + 48)
                    vector.tensor_scalar(
                        out=xc[:], in0=xt[:], scalar1=-1.0, scalar2=1.0,
                        op0=ALU.max, op1=ALU.min,
                    ).then_inc(dve_sem, 1)
                    vector.wait_ge(fsq_sem, 2 * r + 2)
                    vector.tensor_scalar_mul(t1[:], rt[:], SINM).then_inc(dve_sem, 1)
                    vector.wait_ge(dve_sem, 6 * r + 2)
                    vector.tensor_scalar(
                        out=fx[:], in0=xc[:], scalar1=COSM, scalar2=t1[:, :1],
                        op0=ALU.mult, op1=ALU.subtract,
                    ).then_inc(dve_sem, 1)
                    vector.wait_ge(dve_sem, 6 * r + 3)
                    vector.tensor_scalar(
                        out=dl[:], in0=fx[:], scalar1=xc[:, :1], scalar2=None,
                        op0=ALU.subtract,
                    ).then_inc(dve_sem, 1)
                    vector.wait_ge(dve_sem, 6 * r + 4)
                    vector.tensor_scalar(
                        out=sm[:], in0=dl[:], scalar1=own_t[:, :1],
                        scalar2=xc[:, :1], op0=ALU.mult, op1=ALU.add,
                    ).then_inc(dve_sem, 1)
                    vector.wait_ge(dve_sem, 6 * r + 5)
                    vector.tensor_scalar_mul(val[:], sm[:], S).then_inc(dve_sem, 1)

            @block.gpsimd
            def _(gpsimd: bass.BassEngine):
                for r in range(repeat):
                    gpsimd.dma_start(out=idx_t[:], in_=idx[:]).then_inc(fix_sem, 16)
                    gpsimd.dma_start(out=own_t[:], in_=own[:]).then_inc(fix_sem, 16)
                    gpsimd.wait_ge(fix_sem, 48 * r + 32)
                    gpsimd.indirect_dma_start(
                        out=xt[:],
                        out_offset=None,
                        in_=x[:],
                        in_offset=bass.IndirectOffsetOnAxis(ap=idx_t[:, :1], axis=1),
                    ).then_inc(fix_sem, 16)
                    gpsimd.wait_ge(dve_sem, 6 * r + 6)
                    for k in range(NLANES):
                        n_k = len(
                            [g for g in range((r + 1) * n_tiles) if g % NLANES == k]
                        )
                        if n_k:
                            gpsimd.wait_ge(out_sems[k], 16 * n_k)
                    gpsimd.indirect_dma_start(
                        out=y[:],
                        out_offset=bass.IndirectOffsetOnAxis(ap=idx_t[:, :1], axis=1),
                        in_=val[:],
                        in_offset=None,
                    ).then_inc(scat_sem, 16)
                    gpsimd.wait_ge(scat_sem, 16 * (r + 1))

            return nc

        if split_mul:
            n_even = (n_tiles + 1) // 2
            n_odd = n_tiles // 2

            @block.scalar
            def _(scalar: bass.BassEngine):
                for r in range(repeat):
                    for i in range(n_tiles):
                        g = r * n_tiles + i
                        b = bufs[g % nbuf]
                        if i % 2 == 0:  # ACT scales even tiles
                            scalar.wait_ge(in_sems[g % NLANES], 16 * lane_count(g))
                            scalar.mul(
                                b[:, : width(i)], b[:, : width(i)], S
                            ).then_inc(act_sem, 1)
                            scalar.wait_ge(act_sem, n_even * r + i // 2 + 1)
                        else:  # DVE scaled it
                            scalar.wait_ge(dvb_sem, n_odd * r + (i + 1) // 2)
                        scalar.dma_start(
                            out=y[:, col_slice(i)], in_=b[:, : width(i)]
                        ).then_inc(out_sems[g % NLANES], 16)
                    # fixup: sq = xc^2 ; rt = sqrt(1 - sq)
                    scalar.wait_ge(dve_sem, 6 * r + 1)
                    scalar.activation(sq[:], xc[:], ACTF.Square).then_inc(fsq_sem, 1)
                    scalar.wait_ge(fsq_sem, 2 * r + 1)
                    scalar.activation(
                        rt[:], sq[:], ACTF.Sqrt, bias=1.0, scale=-1.0
                    ).then_inc(fsq_sem, 1)

            @block.vector
            def _(vector: bass.BassEngine):
                for r in range(repeat):
                    for i in range(1, n_tiles, 2):
                        g = r * n_tiles + i
                        b = bufs[g % nbuf]
                        vector.wait_ge(in_sems[g % NLANES], 16 * lane_count(g))
                        vector.tensor_scalar_mul(
                            b[:, : width(i)], b[:, : width(i)], S
                        ).then_inc(dvb_sem, 1)
                    # fixup chain (after bulk so it never stalls the muls)
                    vector.wait_ge(fix_sem, 48 * r + 48)
                    vector.tensor_scalar(
                        out=xc[:], in0=xt[:], scalar1=-1.0, scalar2=1.0,
                        op0=ALU.max, op1=ALU.min,
                    ).then_inc(dve_sem, 1)
                    vector.wait_ge(fsq_sem, 2 * r + 2)
                    vector.tensor_scalar_mul(t1[:], rt[:], SINM).then_inc(dve_sem, 1)
                    vector.wait_ge(dve_sem, 6 * r + 2)
                    vector.tensor_scalar(
                        out=fx[:], in0=xc[:], scalar1=COSM, scalar2=t1[:, :1],
                        op0=ALU.mult, op1=ALU.subtract,
                    ).then_inc(dve_sem, 1)
                    vector.wait_ge(dve_sem, 6 * r + 3)
                    vector.tensor_scalar(
                        out=dl[:], in0=fx[:], scalar1=xc[:, :1], scalar2=None,
                        op0=ALU.subtract,
                    ).then_inc(dve_sem, 1)
                    vector.wait_ge(dve_sem, 6 * r + 4)
                    vector.tensor_scalar(
                        out=sm[:], in0=dl[:], scalar1=own_t[:, :1],
                        scalar2=xc[:, :1], op0=ALU.mult, op1=ALU.add,
                    ).then_inc(dve_sem, 1)
                    vector.wait_ge(dve_sem, 6 * r + 5)
                    vector.tensor_scalar_mul(val[:], sm[:], S).then_inc(dve_sem, 1)

            @block.gpsimd
            def _(gpsimd: bass.BassEngine):
                for r in range(repeat):
                    gpsimd.dma_start(out=idx_t[:], in_=idx[:]).then_inc(fix_sem, 16)
                    gpsimd.dma_start(out=own_t[:], in_=own[:]).then_inc(fix_sem, 16)
                    gpsimd.wait_ge(fix_sem, 48 * r + 32)
                    gpsimd.indirect_dma_start(
                        out=xt[:],
                        out_offset=None,
                        in_=x[:],
                        in_offset=bass.IndirectOffsetOnAxis(ap=idx_t[:, :1], axis=1),
                    ).then_inc(fix_sem, 16)
                    gpsimd.wait_ge(dve_sem, 6 * r + 6)
                    for k in range(NLANES):
                        n_k = len(
                            [g for g in range((r + 1) * n_tiles) if g % NLANES == k]
                        )
                        if n_k:
                            gpsimd.wait_ge(out_sems[k], 16 * n_k)
                    gpsimd.indirect_dma_start(
                        out=y[:],
                        out_offset=bass.IndirectOffsetOnAxis(ap=idx_t[:, :1], axis=1),
                        in_=val[:],
                        in_offset=None,
                    ).then_inc(scat_sem, 16)
                    gpsimd.wait_ge(scat_sem, 16 * (r + 1))

            return nc

        @block.scalar
        def _(scalar: bass.BassEngine):
            for r in range(repeat):
                # bulk: y tile = 64 * x tile.  Engines are pipelined, so every
                # same-engine RAW pair also gets an explicit sem sync.  The two
                # fixup ACT ops are tucked in after tile 0's out-DMA so they
                # never stall the pipeline head waiting for the SWDGE gather.
                for i in range(n_tiles):
                    g = r * n_tiles + i
                    scalar.wait_ge(in_sems[g % NLANES], 16 * lane_count(g))
                    b = bufs[g % nbuf]
                    scalar.mul(b[:, : width(i)], b[:, : width(i)], S).then_inc(
                        act_sem, 1
                    )
                    scalar.wait_ge(act_sem, APR * r + 1 + i + (2 if i > 0 else 0))
                    if burst and g % burst == 0:
                        # write-burst waits until its whole read-burst landed
                        # (soft>0: all but the last `soft` tiles; per-tile
                        # in/mul waits still guard each out individually)
                        gl = g + burst - 1 - soft
                        if gl > g:
                            scalar.wait_ge(in_sems[gl % NLANES], 16 * lane_count(gl))
                    scalar.dma_start(
                        out=y[:, col_slice(i)], in_=b[:, : width(i)]
                    ).then_inc(out_sems[g % NLANES], 16)
                    if i == 0:
                        # fixup: sq = xc^2 ; rt = sqrt(1 - sq)
                        scalar.wait_ge(dve_sem, 6 * r + 1)
                        scalar.activation(sq[:], xc[:], ACTF.Square).then_inc(
                            act_sem, 1
                        )
                        scalar.wait_ge(act_sem, APR * r + 2)
                        scalar.activation(
                            rt[:], sq[:], ACTF.Sqrt, bias=1.0, scale=-1.0
                        ).then_inc(act_sem, 1)

        @block.vector
        def _(vector: bass.BassEngine):
            for r in range(repeat):
                # xc = clip(xt, -1, 1)
                vector.wait_ge(fix_sem, 48 * r + 48)
                vector.tensor_scalar(
                    out=xc[:], in0=xt[:], scalar1=-1.0, scalar2=1.0,
                    op0=ALU.max, op1=ALU.min,
                ).then_inc(dve_sem, 1)
                # after ACT's sqrt: fixed = COSM*xc - SINM*rt
                # val = S * (xc + own * (fixed - xc))
                vector.wait_ge(act_sem, APR * r + 3)
                vector.tensor_scalar_mul(t1[:], rt[:], SINM).then_inc(dve_sem, 1)
                vector.wait_ge(dve_sem, 6 * r + 2)
                vector.tensor_scalar(
                    out=fx[:], in0=xc[:], scalar1=COSM, scalar2=t1[:, :1],
                    op0=ALU.mult, op1=ALU.subtract,
                ).then_inc(dve_sem, 1)
                vector.wait_ge(dve_sem, 6 * r + 3)
                vector.tensor_scalar(
                    out=dl[:], in0=fx[:], scalar1=xc[:, :1], scalar2=None,
                    op0=ALU.subtract,
                ).then_inc(dve_sem, 1)
                vector.wait_ge(dve_sem, 6 * r + 4)
                vector.tensor_scalar(
                    out=sm[:], in0=dl[:], scalar1=own_t[:, :1], scalar2=xc[:, :1],
                    op0=ALU.mult, op1=ALU.add,
                ).then_inc(dve_sem, 1)
                vector.wait_ge(dve_sem, 6 * r + 5)
                vector.tensor_scalar_mul(val[:], sm[:], S).then_inc(dve_sem, 1)

        @block.gpsimd
        def _(gpsimd: bass.BassEngine):
            for r in range(repeat):
                gpsimd.dma_start(out=idx_t[:], in_=idx[:]).then_inc(fix_sem, 16)
                gpsimd.dma_start(out=own_t[:], in_=own[:]).then_inc(fix_sem, 16)
                gpsimd.wait_ge(fix_sem, 48 * r + 32)
                # xt[b] = x.flat[idx[b]] (flat element offset: axis=1 -> coef 1)
                gpsimd.indirect_dma_start(
                    out=xt[:],
                    out_offset=None,
                    in_=x[:],
                    in_offset=bass.IndirectOffsetOnAxis(ap=idx_t[:, :1], axis=1),
                ).then_inc(fix_sem, 16)
                # scatter val into label columns, after ALL bulk writes to y
                gpsimd.wait_ge(dve_sem, 6 * r + 6)
                for k in range(NLANES):
                    n_k = len(
                        [g for g in range((r + 1) * n_tiles) if g % NLANES == k]
                    )
                    if n_k:
                        gpsimd.wait_ge(out_sems[k], 16 * n_k)
                gpsimd.indirect_dma_start(
                    out=y[:],
                    out_offset=bass.IndirectOffsetOnAxis(ap=idx_t[:, :1], axis=1),
                    in_=val[:],
                    in_offset=None,
                ).then_inc(scat_sem, 16)
                gpsimd.wait_ge(scat_sem, 16 * (r + 1))

    return nc


_PROG = None

# Chosen by interleaved A/B on the 8-core axon trn2 mesh: 5-tile read bursts
# alternating with 5-tile write bursts (w=5000 cols = 2.56 MB tiles,
# nbuf=10 = the full 200 KB/partition SBUF budget), with soft=1 letting each
# phase start one tile before the previous phase drains.  Phase alternation
# keeps HBM in one transfer direction at a time (read-only streams measure
# 344 GB/s/core, write-only 355, but concurrently mixed R/W only ~316
# effective), and soft=1 hides most of the per-flip drain bubble.
# Measured ~393 us/repeat vs ~399-408 for the continuously-mixed pipeline.
BEST_KW = dict(w=5000, nbuf=10, burst=5, soft=1, widths=[5000] * 25)


def _get_prog() -> bass.Bass:
    global _PROG
    if _PROG is None:
        _PROG = build_program(**BEST_KW)
    return _PROG


def make_in_maps(logits: np.ndarray, labels: np.ndarray) -> list[dict]:
    logits = np.asarray(logits, dtype=np.float32)
    labels = np.asarray(labels).astype(np.int64)
    rows = np.arange(B, dtype=np.int64)
    in_maps = []
    for m in range(NCORES):
        c0 = m * CS
        loc = labels - c0
        ownm = (labels != -1) & (loc >= 0) & (loc < CS)
        col = np.where(ownm, loc, 0)
        flat = (rows * CS + col).astype(np.int32)
        in_maps.append(
            {
                "x": np.ascontiguousarray(logits[:, c0 : c0 + CS]),
                "idx": flat.reshape(B, 1),
                "own": ownm.astype(np.float32).reshape(B, 1),
            }
        )
    return in_maps


def run(logits: np.ndarray, labels: np.ndarray, trace: bool = False):
    """Returns (full_output, BassKernelResults)."""
    in_maps = make_in_maps(logits, labels)
    res = run_bass_kernel_spmd(_get_prog(), in_maps, list(range(NCORES)), trace=trace)
    out = np.concatenate([res.results[m]["y"] for m in range(NCORES)], axis=1)
    return out, res


def kernel(logits: np.ndarray, labels: np.ndarray) -> np.ndarray:
    global _PROG
    try:
        out, _ = run(logits, labels)
    except Exception:
        # One retry on transient device wedges ("mesh desynced" etc.): a
        # fresh program object forces a fresh compile+dispatch path.
        _PROG = None
        out, _ = run(logits, labels)
    return out



# revision 15
# speedup vs baseline: 2.0880x; 1.0130x over previous
"""CombinedMarginLoss (ArcFace branch, m1=1, m2=0.5, m3=0) on 8 Trainium2 cores.

Math: out[b,c] = 64 * logits[b,c] everywhere except the label column of each
row, where out = 64 * cos(arccos(clip(x)) + 0.5).  The trig expands to
x*cos(.5) - sqrt(1-x^2)*sin(.5), so no transcendental sweep is needed: the
bulk of the tensor is a pure scale-by-64 stream, and only the 128 (row, label)
elements need the margin transform.

Sharding (PartialFC style): split num_classes across the 8 cores; each core
streams its [128, 125000] shard through SBUF (DMA in -> x64 on ACT -> DMA out)
and fixes up the label columns it owns with a tiny indirect-DMA
gather/compute/scatter on the side.

The bulk stream runs in alternating 5-tile read / 5-tile write phases
(burst mode, w=5000, nbuf=10, soft=1): HBM sustains ~344-355 GB/s/core in a
single direction but only ~316 GB/s effective when reads and writes are
concurrently mixed, so phase alternation with a one-tile soft handoff at
each flip is ~1.5% faster than the continuously-mixed pipeline
(~393 us vs ~400 us per sweep; serialized-direction floor is ~366 us).

Written in raw Bass (explicit semaphores, standalone wait_ge instructions):
the walrus build in this toolchain rejects any instruction carrying more than
one sync wait, which rules out the Tile scheduler's emitted sync_info.
"""

import math
from contextlib import ExitStack

import numpy as np

try:
    from concourse import bass, mybir
except ImportError:  # repo not on sys.path in a fresh grading dir
    import sys

    sys.path.insert(0, "/opt/trn_rl_repo")
    from concourse import bass, mybir

from concourse.bass_utils import run_bass_kernel_spmd

B = 128
C = 1_000_000
NCORES = 8
CS = C // NCORES  # classes per core
S = 64.0
M2 = 0.5
COSM = math.cos(M2)
SINM = math.sin(M2)
F32 = mybir.dt.float32
I32 = mybir.dt.int32

TILE_W = 12500  # bulk tile width (columns); [128, W] f32 = 6.4 MB per DMA
NBUF = 4
NLANES = 4  # DMA-completion semaphore lanes, round-robin like Tile's DMAHW0-7


def default_widths(cs: int, w: int) -> list[int]:
    """Tile widths with tapered edges: small tiles at the start so the
    out-stream ramps up sooner, and at the end so the tail drains faster."""
    taper = [w // 4, w // 4, w // 2]
    if cs <= 3 * w or w % 4:
        return [min(w, cs - i * w) for i in range((cs + w - 1) // w)]
    body = cs - 2 * w  # one w of taper on each side
    n_body = body // w
    rem = body - n_body * w
    widths = taper + [w] * n_body + ([rem] if rem else []) + taper[::-1]
    assert sum(widths) == cs
    return widths


def build_program(
    cs: int = CS,
    w: int = TILE_W,
    nbuf: int = NBUF,
    repeat: int = 1,
    widths: list[int] | None = None,
    probe: str | None = None,  # None | "copy" (skip mul+fixup) | "read" (in only)
    split_mul: bool = False,  # odd tiles scaled on DVE instead of ACT
    one_ring: bool = False,  # all DMAs on the sync HWDGE ring (R/W bursts)
    burst: int = 0,  # >0: alternate pure-read / pure-write bursts of this many
    #                  tiles (needs nbuf >= 2*burst, n_tiles % burst == 0)
    soft: int = 0,  # burst mode: let the next phase start this many tiles
    #                 before the previous phase fully drains (hides the flip
    #                 bubble at the cost of brief R/W mixing)
    soft_r: int | None = None,  # override soft for the write->read flip
    soft_w: int | None = None,  # override soft for the read->write flip
) -> bass.Bass:
    """repeat>1 replays the whole pipeline back-to-back into the same output
    (benchmarking aid: wall(R)-wall(1) isolates kernel time from dispatch
    overhead).  Cross-repeat races are benign: every repeat writes identical
    values, and the final scatter is ordered after all bulk writes."""
    if widths is None:
        widths = default_widths(cs, w)
    assert sum(widths) == cs and max(widths) <= w
    offsets = [0]
    for wd in widths:
        offsets.append(offsets[-1] + wd)
    n_tiles = len(widths)
    nc = bass.Bass()
    x = nc.declare_dram_parameter("x", [B, cs], F32, isOutput=False)
    idx = nc.declare_dram_parameter("idx", [B, 1], I32, isOutput=False)
    own = nc.declare_dram_parameter("own", [B, 1], F32, isOutput=False)
    y = nc.declare_dram_parameter("y", [B, cs], F32, isOutput=True)

    ALU = mybir.AluOpType
    ACTF = mybir.ActivationFunctionType

    with ExitStack() as ctx:
        bufs = [
            ctx.enter_context(nc.sbuf_tensor(f"buf{k}", [B, w], F32))
            for k in range(nbuf)
        ]
        idx_t = ctx.enter_context(nc.sbuf_tensor("idx_t", [B, 1], I32))
        own_t = ctx.enter_context(nc.sbuf_tensor("own_t", [B, 1], F32))
        xt = ctx.enter_context(nc.sbuf_tensor("xt", [B, 1], F32))
        xc = ctx.enter_context(nc.sbuf_tensor("xc", [B, 1], F32))
        sq = ctx.enter_context(nc.sbuf_tensor("sq", [B, 1], F32))
        rt = ctx.enter_context(nc.sbuf_tensor("rt", [B, 1], F32))
        t1 = ctx.enter_context(nc.sbuf_tensor("t1", [B, 1], F32))
        fx = ctx.enter_context(nc.sbuf_tensor("fx", [B, 1], F32))
        dl = ctx.enter_context(nc.sbuf_tensor("dl", [B, 1], F32))
        sm = ctx.enter_context(nc.sbuf_tensor("sm", [B, 1], F32))
        val = ctx.enter_context(nc.sbuf_tensor("val", [B, 1], F32))

        block = ctx.enter_context(nc.Block())
        in_sems = [
            ctx.enter_context(nc.semaphore(f"in_sem{k}")) for k in range(NLANES)
        ]
        out_sems = [
            ctx.enter_context(nc.semaphore(f"out_sem{k}")) for k in range(NLANES)
        ]
        fix_sem = ctx.enter_context(nc.semaphore("fix_sem"))
        dve_sem = ctx.enter_context(nc.semaphore("dve_sem"))
        act_sem = ctx.enter_context(nc.semaphore("act_sem"))
        scat_sem = ctx.enter_context(nc.semaphore("scat_sem"))
        dvb_sem = ctx.enter_context(nc.semaphore("dvb_sem"))
        fsq_sem = ctx.enter_context(nc.semaphore("fsq_sem"))

        def col_slice(i):
            return slice(offsets[i], offsets[i + 1])

        def width(i):
            return widths[i]

        # in-DMA i signals in_sems[i % NLANES]; the m-th DMA on a lane raises
        # it to 16*(m+1).  Likewise for out-DMAs.
        def lane_count(i):
            return i // NLANES + 1

        APR = 2 + n_tiles  # act_sem increments per repeat

        if burst:
            assert nbuf >= 2 * burst and n_tiles % burst == 0

        if probe in ("read2", "write2"):
            # One direction only, split across BOTH HWDGE rings (sync=SP ring
            # even tiles, scalar=ACT ring odd tiles).  If a single ring is
            # the ~344 GB/s cap, this should beat the 1-ring read/write
            # probes; if the HBM bus is the cap, it will match them.
            half = nbuf // 2
            G = repeat * n_tiles

            def gen(eng, e):
                ks = [g for g in range(G) if (g % n_tiles) % 2 == e]
                for k, g in enumerate(ks):
                    i = g % n_tiles
                    lane = 2 * e + (k % 2)
                    if probe == "read2":
                        if k >= half:
                            kp = k - half
                            eng.wait_ge(
                                in_sems[2 * e + kp % 2], 16 * (kp // 2 + 1)
                            )
                        eng.dma_start(
                            out=bufs[e * half + k % half][:, : width(i)],
                            in_=x[:, col_slice(i)],
                        ).then_inc(in_sems[lane], 16)
                    else:
                        eng.dma_start(
                            out=y[:, col_slice(i)],
                            in_=bufs[e * half + k % half][:, : width(i)],
                        ).then_inc(in_sems[lane], 16)
                for off in (0, 1):
                    n_l = len([k for k in range(len(ks)) if k % 2 == off])
                    if n_l:
                        eng.wait_ge(in_sems[2 * e + off], 16 * n_l)

            @block.sync
            def _(sync: bass.BassEngine):
                gen(sync, 0)

            @block.scalar
            def _(scalar: bass.BassEngine):
                gen(scalar, 1)

            return nc

        if probe == "write":
            # Pure write stream: y tiles from (uninitialized) SBUF buffers.
            # Buffers are read-only sources, so no recycle waits are needed;
            # only a final drain before program end.  No in-DMAs at all.
            @block.scalar
            def _(scalar: bass.BassEngine):
                G = repeat * n_tiles
                for g in range(G):
                    i = g % n_tiles
                    scalar.dma_start(
                        out=y[:, col_slice(i)], in_=bufs[g % nbuf][:, : width(i)]
                    ).then_inc(out_sems[g % NLANES], 16)
                for k in range(NLANES):
                    n_k = len([g for g in range(G) if g % NLANES == k])
                    if n_k:
                        scalar.wait_ge(out_sems[k], 16 * n_k)

            return nc

        @block.sync
        def _(sync: bass.BassEngine):
            for g in range(repeat * n_tiles):
                i = g % n_tiles
                if g >= nbuf:
                    j = g - nbuf  # previous tenant of this buffer
                    recycle = in_sems if probe == "read" else out_sems
                    sync.wait_ge(recycle[j % NLANES], 16 * lane_count(j))
                if burst and g % burst == 0 and g >= burst:
                    # read-burst b starts only after write-burst b-1 drained
                    # (with soft>0: after all but `soft` of its tiles drained)
                    gw = g - 1 - (soft if soft_r is None else soft_r)
                    if gw >= 0:
                        sync.wait_ge(out_sems[gw % NLANES], 16 * lane_count(gw))
                sync.dma_start(
                    out=bufs[g % nbuf][:, : width(i)], in_=x[:, col_slice(i)]
                ).then_inc(in_sems[g % NLANES], 16)
            if probe == "read":  # drain before program end
                G = repeat * n_tiles
                for k in range(NLANES):
                    n_k = len([g for g in range(G) if g % NLANES == k])
                    if n_k:
                        sync.wait_ge(in_sems[k], 16 * n_k)

        if probe == "read":
            return nc

        if probe == "serial":
            # All DMAs on ONE HWDGE ring (sync), alternating whole-tile reads
            # and writes: in0,in1,out0,in2,out1,...  Probes whether avoiding
            # concurrent R/W streams beats the two-ring copy skeleton.  Pure
            # copy is per-SDMA-engine FIFO-safe without in-completion waits.
            @block.sync
            def _(sync: bass.BassEngine):
                G = repeat * n_tiles
                lead = 2
                for g in range(G + lead):
                    if g < G:
                        i = g % n_tiles
                        if g >= nbuf:
                            j = g - nbuf
                            sync.wait_ge(out_sems[j % NLANES], 16 * lane_count(j))
                        sync.dma_start(
                            out=bufs[g % nbuf][:, : width(i)], in_=x[:, col_slice(i)]
                        ).then_inc(in_sems[g % NLANES], 16)
                    if g >= lead:
                        k = g - lead
                        i = k % n_tiles
                        sync.dma_start(
                            out=y[:, col_slice(i)], in_=bufs[k % nbuf][:, : width(i)]
                        ).then_inc(out_sems[k % NLANES], 16)
                for kk in range(NLANES):
                    n_k = len([g for g in range(G) if g % NLANES == kk])
                    if n_k:
                        sync.wait_ge(out_sems[kk], 16 * n_k)

            return nc

        if probe == "copy":

            @block.scalar
            def _(scalar: bass.BassEngine):
                for r in range(repeat):
                    for i in range(n_tiles):
                        g = r * n_tiles + i
                        scalar.wait_ge(in_sems[g % NLANES], 16 * lane_count(g))
                        scalar.dma_start(
                            out=y[:, col_slice(i)], in_=bufs[g % nbuf][:, : width(i)]
                        ).then_inc(out_sems[g % NLANES], 16)

            return nc

        if one_ring:
            G = repeat * n_tiles
            lead = 2

            @block.sync
            def _(sync: bass.BassEngine):
                # Single HWDGE ring serializes whole-tile reads and writes so
                # HBM never interleaves R/W at packet granularity.  out(k)
                # gates on mul(k) (act_sem is one inc per bulk mul).
                for g in range(G + lead):
                    if g < G:
                        i = g % n_tiles
                        if g >= nbuf:
                            j = g - nbuf
                            sync.wait_ge(out_sems[j % NLANES], 16 * lane_count(j))
                        sync.dma_start(
                            out=bufs[g % nbuf][:, : width(i)], in_=x[:, col_slice(i)]
                        ).then_inc(in_sems[g % NLANES], 16)
                    if g >= lead:
                        k = g - lead
                        i = k % n_tiles
                        sync.wait_ge(act_sem, k + 1)
                        sync.dma_start(
                            out=y[:, col_slice(i)], in_=bufs[k % nbuf][:, : width(i)]
                        ).then_inc(out_sems[k % NLANES], 16)

            @block.scalar
            def _(scalar: bass.BassEngine):
                for r in range(repeat):
                    for i in range(n_tiles):
                        g = r * n_tiles + i
                        scalar.wait_ge(in_sems[g % NLANES], 16 * lane_count(g))
                        b = bufs[g % nbuf]
                        scalar.mul(b[:, : width(i)], b[:, : width(i)], S).then_inc(
                            act_sem, 1
                        )
                    # fixup: sq = xc^2 ; rt = sqrt(1 - sq)
                    scalar.wait_ge(dve_sem, 6 * r + 1)
                    scalar.activation(sq[:], xc[:], ACTF.Square).then_inc(fsq_sem, 1)
                    scalar.wait_ge(fsq_sem, 2 * r + 1)
                    scalar.activation(
                        rt[:], sq[:], ACTF.Sqrt, bias=1.0, scale=-1.0
                    ).then_inc(fsq_sem, 1)

            @block.vector
            def _(vector: bass.BassEngine):
                for r in range(repeat):
                    vector.wait_ge(fix_sem, 48 * r + 48)
                    vector.tensor_scalar(
                        out=xc[:], in0=xt[:], scalar1=-1.0, scalar2=1.0,
                        op0=ALU.max, op1=ALU.min,
                    ).then_inc(dve_sem, 1)
                    vector.wait_ge(fsq_sem, 2 * r + 2)
                    vector.tensor_scalar_mul(t1[:], rt[:], SINM).then_inc(dve_sem, 1)
                    vector.wait_ge(dve_sem, 6 * r + 2)
                    vector.tensor_scalar(
                        out=fx[:], in0=xc[:], scalar1=COSM, scalar2=t1[:, :1],
                        op0=ALU.mult, op1=ALU.subtract,
                    ).then_inc(dve_sem, 1)
                    vector.wait_ge(dve_sem, 6 * r + 3)
                    vector.tensor_scalar(
                        out=dl[:], in0=fx[:], scalar1=xc[:, :1], scalar2=None,
                        op0=ALU.subtract,
                    ).then_inc(dve_sem, 1)
                    vector.wait_ge(dve_sem, 6 * r + 4)
                    vector.tensor_scalar(
                        out=sm[:], in0=dl[:], scalar1=own_t[:, :1],
                        scalar2=xc[:, :1], op0=ALU.mult, op1=ALU.add,
                    ).then_inc(dve_sem, 1)
                    vector.wait_ge(dve_sem, 6 * r + 5)
                    vector.tensor_scalar_mul(val[:], sm[:], S).then_inc(dve_sem, 1)

            @block.gpsimd
            def _(gpsimd: bass.BassEngine):
                for r in range(repeat):
                    gpsimd.dma_start(out=idx_t[:], in_=idx[:]).then_inc(fix_sem, 16)
                    gpsimd.dma_start(out=own_t[:], in_=own[:]).then_inc(fix_sem, 16)
                    gpsimd.wait_ge(fix_sem, 48 * r + 32)
                    gpsimd.indirect_dma_start(
                        out=xt[:],
                        out_offset=None,
                        in_=x[:],
                        in_offset=bass.IndirectOffsetOnAxis(ap=idx_t[:, :1], axis=1),
                    ).then_inc(fix_sem, 16)
                    gpsimd.wait_ge(dve_sem, 6 * r + 6)
                    for k in range(NLANES):
                        n_k = len(
                            [g for g in range((r + 1) * n_tiles) if g % NLANES == k]
                        )
                        if n_k:
                            gpsimd.wait_ge(out_sems[k], 16 * n_k)
                    gpsimd.indirect_dma_start(
                        out=y[:],
                        out_offset=bass.IndirectOffsetOnAxis(ap=idx_t[:, :1], axis=1),
                        in_=val[:],
                        in_offset=None,
                    ).then_inc(scat_sem, 16)
                    gpsimd.wait_ge(scat_sem, 16 * (r + 1))

            return nc

        if split_mul:
            n_even = (n_tiles + 1) // 2
            n_odd = n_tiles // 2

            @block.scalar
            def _(scalar: bass.BassEngine):
                for r in range(repeat):
                    for i in range(n_tiles):
                        g = r * n_tiles + i
                        b = bufs[g % nbuf]
                        if i % 2 == 0:  # ACT scales even tiles
                            scalar.wait_ge(in_sems[g % NLANES], 16 * lane_count(g))
                            scalar.mul(
                                b[:, : width(i)], b[:, : width(i)], S
                            ).then_inc(act_sem, 1)
                            scalar.wait_ge(act_sem, n_even * r + i // 2 + 1)
                        else:  # DVE scaled it
                            scalar.wait_ge(dvb_sem, n_odd * r + (i + 1) // 2)
                        scalar.dma_start(
                            out=y[:, col_slice(i)], in_=b[:, : width(i)]
                        ).then_inc(out_sems[g % NLANES], 16)
                    # fixup: sq = xc^2 ; rt = sqrt(1 - sq)
                    scalar.wait_ge(dve_sem, 6 * r + 1)
                    scalar.activation(sq[:], xc[:], ACTF.Square).then_inc(fsq_sem, 1)
                    scalar.wait_ge(fsq_sem, 2 * r + 1)
                    scalar.activation(
                        rt[:], sq[:], ACTF.Sqrt, bias=1.0, scale=-1.0
                    ).then_inc(fsq_sem, 1)

            @block.vector
            def _(vector: bass.BassEngine):
                for r in range(repeat):
                    for i in range(1, n_tiles, 2):
                        g = r * n_tiles + i
                        b = bufs[g % nbuf]
                        vector.wait_ge(in_sems[g % NLANES], 16 * lane_count(g))
                        vector.tensor_scalar_mul(
                            b[:, : width(i)], b[:, : width(i)], S
                        ).then_inc(dvb_sem, 1)
                    # fixup chain (after bulk so it never stalls the muls)
                    vector.wait_ge(fix_sem, 48 * r + 48)
                    vector.tensor_scalar(
                        out=xc[:], in0=xt[:], scalar1=-1.0, scalar2=1.0,
                        op0=ALU.max, op1=ALU.min,
                    ).then_inc(dve_sem, 1)
                    vector.wait_ge(fsq_sem, 2 * r + 2)
                    vector.tensor_scalar_mul(t1[:], rt[:], SINM).then_inc(dve_sem, 1)
                    vector.wait_ge(dve_sem, 6 * r + 2)
                    vector.tensor_scalar(
                        out=fx[:], in0=xc[:], scalar1=COSM, scalar2=t1[:, :1],
                        op0=ALU.mult, op1=ALU.subtract,
                    ).then_inc(dve_sem, 1)
                    vector.wait_ge(dve_sem, 6 * r + 3)
                    vector.tensor_scalar(
                        out=dl[:], in0=fx[:], scalar1=xc[:, :1], scalar2=None,
                        op0=ALU.subtract,
                    ).then_inc(dve_sem, 1)
                    vector.wait_ge(dve_sem, 6 * r + 4)
                    vector.tensor_scalar(
                        out=sm[:], in0=dl[:], scalar1=own_t[:, :1],
                        scalar2=xc[:, :1], op0=ALU.mult, op1=ALU.add,
                    ).then_inc(dve_sem, 1)
                    vector.wait_ge(dve_sem, 6 * r + 5)
                    vector.tensor_scalar_mul(val[:], sm[:], S).then_inc(dve_sem, 1)

            @block.gpsimd
            def _(gpsimd: bass.BassEngine):
                for r in range(repeat):
                    gpsimd.dma_start(out=idx_t[:], in_=idx[:]).then_inc(fix_sem, 16)
                    gpsimd.dma_start(out=own_t[:], in_=own[:]).then_inc(fix_sem, 16)
                    gpsimd.wait_ge(fix_sem, 48 * r + 32)
                    gpsimd.indirect_dma_start(
                        out=xt[:],
                        out_offset=None,
                        in_=x[:],
                        in_offset=bass.IndirectOffsetOnAxis(ap=idx_t[:, :1], axis=1),
                    ).then_inc(fix_sem, 16)
                    gpsimd.wait_ge(dve_sem, 6 * r + 6)
                    for k in range(NLANES):
                        n_k = len(
                            [g for g in range((r + 1) * n_tiles) if g % NLANES == k]
                        )
                        if n_k:
                            gpsimd.wait_ge(out_sems[k], 16 * n_k)
                    gpsimd.indirect_dma_start(
                        out=y[:],
                        out_offset=bass.IndirectOffsetOnAxis(ap=idx_t[:, :1], axis=1),
                        in_=val[:],
                        in_offset=None,
                    ).then_inc(scat_sem, 16)
                    gpsimd.wait_ge(scat_sem, 16 * (r + 1))

            return nc

        @block.scalar
        def _(scalar: bass.BassEngine):
            for r in range(repeat):
                # bulk: y tile = 64 * x tile.  Engines are pipelined, so every
                # same-engine RAW pair also gets an explicit sem sync.  The two
                # fixup ACT ops are tucked in after tile 0's out-DMA so they
                # never stall the pipeline head waiting for the SWDGE gather.
                for i in range(n_tiles):
                    g = r * n_tiles + i
                    scalar.wait_ge(in_sems[g % NLANES], 16 * lane_count(g))
                    b = bufs[g % nbuf]
                    scalar.mul(b[:, : width(i)], b[:, : width(i)], S).then_inc(
                        act_sem, 1
                    )
                    scalar.wait_ge(act_sem, APR * r + 1 + i + (2 if i > 0 else 0))
                    if burst and g % burst == 0:
                        # write-burst waits until its whole read-burst landed
                        # (soft>0: all but the last `soft` tiles; per-tile
                        # in/mul waits still guard each out individually)
                        gl = g + burst - 1 - (soft if soft_w is None else soft_w)
                        if gl > g:
                            scalar.wait_ge(in_sems[gl % NLANES], 16 * lane_count(gl))
                    scalar.dma_start(
                        out=y[:, col_slice(i)], in_=b[:, : width(i)]
                    ).then_inc(out_sems[g % NLANES], 16)
                    if i == 0:
                        # fixup: sq = xc^2 ; rt = sqrt(1 - sq)
                        scalar.wait_ge(dve_sem, 6 * r + 1)
                        scalar.activation(sq[:], xc[:], ACTF.Square).then_inc(
                            act_sem, 1
                        )
                        scalar.wait_ge(act_sem, APR * r + 2)
                        scalar.activation(
                            rt[:], sq[:], ACTF.Sqrt, bias=1.0, scale=-1.0
                        ).then_inc(act_sem, 1)

        @block.vector
        def _(vector: bass.BassEngine):
            for r in range(repeat):
                # xc = clip(xt, -1, 1)
                vector.wait_ge(fix_sem, 48 * r + 48)
                vector.tensor_scalar(
                    out=xc[:], in0=xt[:], scalar1=-1.0, scalar2=1.0,
                    op0=ALU.max, op1=ALU.min,
                ).then_inc(dve_sem, 1)
                # after ACT's sqrt: fixed = COSM*xc - SINM*rt
                # val = S * (xc + own * (fixed - xc))
                vector.wait_ge(act_sem, APR * r + 3)
                vector.tensor_scalar_mul(t1[:], rt[:], SINM).then_inc(dve_sem, 1)
                vector.wait_ge(dve_sem, 6 * r + 2)
                vector.tensor_scalar(
                    out=fx[:], in0=xc[:], scalar1=COSM, scalar2=t1[:, :1],
                    op0=ALU.mult, op1=ALU.subtract,
                ).then_inc(dve_sem, 1)
                vector.wait_ge(dve_sem, 6 * r + 3)
                vector.tensor_scalar(
                    out=dl[:], in0=fx[:], scalar1=xc[:, :1], scalar2=None,
                    op0=ALU.subtract,
                ).then_inc(dve_sem, 1)
                vector.wait_ge(dve_sem, 6 * r + 4)
                vector.tensor_scalar(
                    out=sm[:], in0=dl[:], scalar1=own_t[:, :1], scalar2=xc[:, :1],
                    op0=ALU.mult, op1=ALU.add,
                ).then_inc(dve_sem, 1)
                vector.wait_ge(dve_sem, 6 * r + 5)
                vector.tensor_scalar_mul(val[:], sm[:], S).then_inc(dve_sem, 1)

        @block.gpsimd
        def _(gpsimd: bass.BassEngine):
            for r in range(repeat):
                gpsimd.dma_start(out=idx_t[:], in_=idx[:]).then_inc(fix_sem, 16)
                gpsimd.dma_start(out=own_t[:], in_=own[:]).then_inc(fix_sem, 16)
                gpsimd.wait_ge(fix_sem, 48 * r + 32)
                # xt[b] = x.flat[idx[b]] (flat element offset: axis=1 -> coef 1)
                gpsimd.indirect_dma_start(
                    out=xt[:],
                    out_offset=None,
                    in_=x[:],
                    in_offset=bass.IndirectOffsetOnAxis(ap=idx_t[:, :1], axis=1),
                ).then_inc(fix_sem, 16)
                # scatter val into label columns, after ALL bulk writes to y
                gpsimd.wait_ge(dve_sem, 6 * r + 6)
                for k in range(NLANES):
                    n_k = len(
                        [g for g in range((r + 1) * n_tiles) if g % NLANES == k]
                    )
                    if n_k:
                        gpsimd.wait_ge(out_sems[k], 16 * n_k)
                gpsimd.indirect_dma_start(
                    out=y[:],
                    out_offset=bass.IndirectOffsetOnAxis(ap=idx_t[:, :1], axis=1),
                    in_=val[:],
                    in_offset=None,
                ).then_inc(scat_sem, 16)
                gpsimd.wait_ge(scat_sem, 16 * (r + 1))

    return nc


_PROG = None

# Chosen by interleaved A/B on the 8-core axon trn2 mesh: 4-tile read bursts
# alternating with 4-tile write bursts (w=6250 cols = 3.2 MB tiles, nbuf=8 =
# the full 200 KB/partition SBUF budget), with soft=1 letting each phase
# start one tile before the previous phase drains.  Phase alternation keeps
# HBM in one transfer direction at a time (read-only streams measure
# 344 GB/s/core, write-only 355, but concurrently mixed R/W only ~316
# effective), and soft=1 hides most of the per-flip drain bubble.
# Measured ~386 us/repeat vs ~393 for 5x5000 phases and ~399-408 for the
# continuously-mixed pipeline (serialized-direction floor ~366 us).
BEST_KW = dict(w=6250, nbuf=8, burst=4, soft=1, widths=[6250] * 20)


def _get_prog() -> bass.Bass:
    global _PROG
    if _PROG is None:
        _PROG = build_program(**BEST_KW)
    return _PROG


def make_in_maps(logits: np.ndarray, labels: np.ndarray) -> list[dict]:
    logits = np.asarray(logits, dtype=np.float32)
    labels = np.asarray(labels).astype(np.int64)
    rows = np.arange(B, dtype=np.int64)
    in_maps = []
    for m in range(NCORES):
        c0 = m * CS
        loc = labels - c0
        ownm = (labels != -1) & (loc >= 0) & (loc < CS)
        col = np.where(ownm, loc, 0)
        flat = (rows * CS + col).astype(np.int32)
        in_maps.append(
            {
                "x": np.ascontiguousarray(logits[:, c0 : c0 + CS]),
                "idx": flat.reshape(B, 1),
                "own": ownm.astype(np.float32).reshape(B, 1),
            }
        )
    return in_maps


def run(logits: np.ndarray, labels: np.ndarray, trace: bool = False):
    """Returns (full_output, BassKernelResults)."""
    in_maps = make_in_maps(logits, labels)
    res = run_bass_kernel_spmd(_get_prog(), in_maps, list(range(NCORES)), trace=trace)
    out = np.concatenate([res.results[m]["y"] for m in range(NCORES)], axis=1)
    return out, res


def kernel(logits: np.ndarray, labels: np.ndarray) -> np.ndarray:
    global _PROG
    try:
        out, _ = run(logits, labels)
    except Exception:
        # One retry on transient device wedges ("mesh desynced" etc.): a
        # fresh program object forces a fresh compile+dispatch path.
        _PROG = None
        out, _ = run(logits, labels)
    return out

